# revision 67
# baseline (speedup 1.0000x reference)
"""Trainium2 Bass kernel for a 2-layer BCos-GCN (nn_BCosGCN_28346784153649).

Strategy (8 NeuronCores, SPMD), v3:

  GCN layer: out = dinv_dst * ((S @ T + T_own) @ W) + b,  T = dinv .* feat
  (S is the pure 0/1 edge one-hot; dinv_src is folded into the table rows,
  the self-loop folds into the same PSUM-accumulated sum.)

  v3 changes over v2 (1409us baseline):
  - No DRAM staging of the T1 tables: gathers read the replicated kernel
    inputs directly (-150us prologue).
  - One-hot S chunks are precomputed on the HOST and streamed from DRAM as
    fp8 (kills ~500us of DVE is_equal work); same for the mean-pool
    one-hots.
  - Self-loop row T_own^T is added via an extra PSUM-accumulating matmul
    with an identity moving operand (kills the DVE add + the layer-2 PE
    transposes + scalar copies).
  - Epilogue: LayerNorm statistics via scalar-engine Copy/Square with
    accum_out (sum + sum-of-squares as free by-products), rsqrt via
    Exp(-0.5*Ln(x)) so every activation lives in the single
    natural_log_exp_and_others table set (no ACT_TABLE_LOAD churn), and the
    dinv_dst scale is skipped entirely when bias==0 (LayerNorm is
    scale-invariant per row).
  - Layer-2 AllGather: Shared-output collectives, split into 4 row-chunks
    per residue bank, fired as soon as their groups finish so only a small
    tail blocks layer 2.

Host-side preprocessing (cached GCN normalization dinv, node placement,
edge bucketing/padding, index tables, one-hot tables) is numpy; all heavy
per-edge/per-node compute runs on the NeuronCores.
"""

import os
import sys

sys.path.insert(0, "/opt/trn_rl_repo")

import numpy as np

from concourse import bacc, tile, mybir
from concourse.bass_utils import run_bass_kernel_spmd
from concourse.masks import make_identity

# ---------------------------------------------------------------- constants
N, E, F, H, C, G = 100000, 1600000, 128, 128, 10, 512
LN_EPS = 1e-5
BCOS_EPS = 1e-6
TEMP = 1.5
RR = 0.6  # residual ratio; bcos exponent B == 1.0 -> bcos(h) = TEMP*h/(nrm+eps)

NCORES = 8
P = 128
REAL_PER_CORE = N // NCORES            # 12500
NODES_PER_CORE = 12800                 # padded: 100 blocks of 128
BLOCKS_PER_CORE = NODES_PER_CORE // P  # 100
NPAD = NODES_PER_CORE * NCORES         # 102400
NBLK = NPAD // P                       # 800
RES = 4                                # residue banks (slot>>5)
B_GRP = 4                              # dst blocks per group / PSUM tile
N_GRP = BLOCKS_PER_CORE // B_GRP       # 25 groups per core
ROWS_PER_BANK = NPAD // RES            # 25600 (< int16 max)

# AllGather row-chunks (block ranges) and the group after which each fires.
# Chunk q covers blocks [QBLK[q][0], QBLK[q][1]) of every core; it is ready
# once groups < FIRE_AT_G[q] have finished (fire point g means: called after
# agg_mm(g), before finish(g-1), so finishes 0..g-2 are done).
_AG_CHUNKS = int(os.environ.get("BASS_AG_CHUNKS", "1"))
if _AG_CHUNKS == 1:
    QBLK = [(0, 100)]
    FIRE_AT_G = [None]
elif _AG_CHUNKS == 2:
    QBLK = [(0, 48), (48, 100)]
    FIRE_AT_G = [13, None]
else:
    QBLK = [(0, 24), (24, 48), (48, 76), (76, 100)]
    FIRE_AT_G = [7, 13, 20, None]      # None -> after the loop
QROWS = [(b1 - b0) * 32 for b0, b1 in QBLK]       # per-core rows per chunk
QBASE = [0]
for _q in range(1, len(QBLK)):
    QBASE.append(QBASE[-1] + NCORES * QROWS[_q - 1])
QUARTER_OF_GROUP = []
for _g in range(N_GRP):
    _b = _g * B_GRP
    for _qi, (_b0, _b1) in enumerate(QBLK):
        if _b0 <= _b < _b1:
            QUARTER_OF_GROUP.append(_qi)
            break

F16 = mybir.dt.float16
F32 = mybir.dt.float32
F8 = mybir.dt.float8e4
I16 = mybir.dt.int16
I32 = mybir.dt.int32
AOp = mybir.AluOpType
Act = mybir.ActivationFunctionType
AxX = mybir.AxisListType.X


# ---------------------------------------------------------------- host prep
def _lpt_blocks(indeg_core: np.ndarray) -> list[list[int]]:
    """Pack the core's real nodes into 100 blocks of <=128, balancing the
    in-degree sum per block (greedy LPT)."""
    import heapq

    order = np.argsort(-indeg_core, kind="stable")
    heap = [(0, 0, b) for b in range(BLOCKS_PER_CORE)]
    heapq.heapify(heap)
    blocks: list[list[int]] = [[] for _ in range(BLOCKS_PER_CORE)]
    for v in order:
        while True:
            load, cnt, b = heapq.heappop(heap)
            if cnt < P:
                break
        blocks[b].append(int(v))
        heapq.heappush(heap, (load + int(indeg_core[v]), cnt + 1, b))
    return blocks


def _color_banks(ownblk, src, dstblk, rounds=24, seed=0):
    """Greedy residue-bank coloring balancing (dst-block, color) edge cells
    at <=512 (-> K=4), subject to <=32 nodes per (own-block, color)."""
    SLOT_CAP = P // RES
    Nn = ownblk.shape[0]
    rng = np.random.default_rng(seed)
    eorder = np.argsort(src, kind="stable")
    e_dstblk = dstblk[eorder]
    esrc = src[eorder]
    degn = np.bincount(src, minlength=Nn)
    estart = np.concatenate([[0], np.cumsum(degn)])
    cellcnt = np.zeros((NBLK, RES), np.int64)
    slotcnt = np.zeros((NBLK, RES), np.int32)
    color = np.full(Nn, -1, np.int32)
    order = np.argsort(-degn, kind="stable")
    target = max(1.0, dstblk.shape[0] / (NBLK * RES))
    cap = int(np.ceil(target / P) * P)
    for bt in np.array_split(order, rounds):
        nb = bt.shape[0]
        reps = degn[bt]
        node_rep = np.repeat(np.arange(nb), reps)
        eidx = (np.concatenate([np.arange(estart[v], estart[v + 1]) for v in bt])
                if nb else np.empty(0, np.int64))
        score = np.zeros((nb, RES), np.float64)
        if eidx.size:
            np.add.at(score, node_rep, cellcnt[e_dstblk[eidx]])
        own = ownblk[bt]
        score += np.where(slotcnt[own] >= SLOT_CAP, 1e12, 0.0)
        if eidx.size:
            np.add.at(score, node_rep,
                      np.where(cellcnt[e_dstblk[eidx]] >= cap - 1, 1e6, 0.0))
        score += rng.random((nb, RES))
        ch = np.argmin(score, axis=1).astype(np.int32)
        for i in range(nb):
            o, c = own[i], ch[i]
            if slotcnt[o, c] >= SLOT_CAP:
                c = int(np.argmin(slotcnt[o] + np.where(
                    slotcnt[o] >= SLOT_CAP, 10**9, 0)))
                ch[i] = c
            slotcnt[o, c] += 1
        color[bt] = ch
        if eidx.size:
            np.add.at(cellcnt, (e_dstblk[eidx], ch[node_rep]), 1)
    # exact repair: move nodes out of over-cap cells
    border = np.argsort(e_dstblk, kind="stable")
    bcnt = np.bincount(e_dstblk, minlength=NBLK)
    bstart = np.concatenate([[0], np.cumsum(bcnt)])
    for _ in range(40):
        over = np.argwhere(cellcnt > cap)
        if over.size == 0:
            break
        for bb, cc in over:
            while cellcnt[bb, cc] > cap:
                cands = np.unique(esrc[border[bstart[bb]:bstart[bb + 1]]])
                cands = cands[color[cands] == cc]
                moved = False
                contrib = np.array([
                    np.count_nonzero(e_dstblk[estart[v]:estart[v + 1]] == bb)
                    for v in cands])
                for v in cands[np.argsort(contrib)]:
                    o = ownblk[v]
                    blks = e_dstblk[estart[v]:estart[v + 1]]
                    for c2 in np.argsort(cellcnt[bb]):
                        if c2 == cc or slotcnt[o, c2] >= SLOT_CAP:
                            continue
                        add = np.bincount(blks, minlength=NBLK)
                        touched = np.nonzero(add)[0]
                        if (cellcnt[touched, c2] + add[touched] <= cap).all():
                            cellcnt[touched, cc] -= add[touched]
                            cellcnt[touched, c2] += add[touched]
                            slotcnt[o, cc] -= 1
                            slotcnt[o, c2] += 1
                            color[v] = c2
                            moved = True
                            break
                    if moved:
                        break
                if not moved:
                    break
    return color


def _pairs():
    ps = [(2 * i, 2 * i + 1) for i in range(N_GRP // 2)]
    if N_GRP % 2:
        ps.append((N_GRP - 1,))
    return ps


def _quarter_of_block(blk_local):
    """Vectorized: local block index -> AG chunk index."""
    q = np.zeros_like(blk_local)
    for qi, (b0, b1) in enumerate(QBLK):
        q = np.where((blk_local >= b0) & (blk_local < b1), qi, q)
    return q


def _bank_row(core_a, blk_a, slot_a):
    """Row of a node inside its residue bank (chunk-major layout)."""
    qi = _quarter_of_block(blk_a)
    base = np.array(QBASE, np.int64)[qi]
    rows = np.array(QROWS, np.int64)[qi]
    b0 = np.array([b[0] for b in QBLK], np.int64)[qi]
    return base + core_a * rows + (blk_a - b0) * 32 + (slot_a & 31)


def _prep(x, src, dst, batch, W1, b1, ln1_w, ln1_b, W2, b2, ln2_w, ln2_b,
          cls_v, cls_g, cls_b, seed=0):
    F8NP = mybir.dt.np(F8)
    indeg = np.bincount(dst, minlength=N)
    deg = indeg.astype(np.float32) + 1.0
    dinv = (1.0 / np.sqrt(deg)).astype(np.float32)

    # ---- node -> (core, block); LPT balance in-degree per block
    ownblk = np.zeros(N, np.int64)
    core_blocks = []
    g_base = np.zeros(NCORES, np.int64)
    for c in range(NCORES):
        lo, hi = c * REAL_PER_CORE, (c + 1) * REAL_PER_CORE
        g_base[c] = int(batch[lo])
        span = int(batch[hi - 1]) - g_base[c]
        assert span < P, f"core {c} spans {span + 1} graphs > 128"
        blocks = _lpt_blocks(indeg[lo:hi])
        core_blocks.append(blocks)
        for b in range(BLOCKS_PER_CORE):
            for v_local in blocks[b]:
                ownblk[lo + v_local] = c * BLOCKS_PER_CORE + b

    # ---- residue-bank coloring (cells <= 512 -> K=4); slot assignment
    s64 = src.astype(np.int64)
    d64 = dst.astype(np.int64)
    color = _color_banks(ownblk, s64, ownblk[d64])
    pos = np.full(N, -1, np.int64)
    for c in range(NCORES):
        lo = c * REAL_PER_CORE
        for b in range(BLOCKS_PER_CORE):
            blk = core_blocks[c][b]
            base = c * NODES_PER_CORE + b * P
            # color r occupies contiguous slots [32r, 32r+31] so each
            # residue class is a contiguous partition range
            nxt = [0, 0, 0, 0]
            for v_local in blk:
                cc = int(color[lo + v_local])
                sl = 32 * cc + nxt[cc]
                nxt[cc] += 1
                pos[lo + v_local] = base + sl
    assert (pos >= 0).all()

    # global padded position -> (core, local block, slot)
    pcore = pos // NODES_PER_CORE
    pblk = (pos % NODES_PER_CORE) // P
    pslot = pos % P

    node_row = _bank_row(pcore, pblk, pslot)         # row within its bank
    node_res = pslot >> 5

    # ---- per-position node data
    node_at = np.full(NPAD, -1, np.int64)
    node_at[pos] = np.arange(N)

    x16 = (x * dinv[:, None]).astype(np.float16)     # T1 rows = dinv .* x

    # replicated residue-bank tables of T1
    xtab = np.zeros((RES, ROWS_PER_BANK, F), np.float16)
    xtab[node_res, node_row] = x16

    # per-core own rows in natural [slot, block, feat] layout, per-(slot,
    # block) dinv, per-node mean-pool one-hot
    xs2 = np.zeros((NCORES, P, BLOCKS_PER_CORE, F), np.float16)
    d1t = np.ones((NCORES, P, BLOCKS_PER_CORE), np.float32)
    pone = np.zeros((NCORES, P, BLOCKS_PER_CORE * P), F8NP)
    for c in range(NCORES):
        sel = node_at[c * NODES_PER_CORE:(c + 1) * NODES_PER_CORE]
        ok = sel >= 0
        xs_flat = np.zeros((NODES_PER_CORE, F), np.float16)
        xs_flat[ok] = x16[sel[ok]]
        xs2[c] = xs_flat.reshape(BLOCKS_PER_CORE, P, F).transpose(1, 0, 2)
        d1 = np.ones(NODES_PER_CORE, np.float32)
        d1[ok] = dinv[sel[ok]]
        d1t[c] = d1.reshape(BLOCKS_PER_CORE, P).T
        # pool one-hot: [slot, block*128 + graph_label]; padded slots -> 0
        pidx = np.nonzero(ok)[0]
        pb = pidx // P
        psl = pidx % P
        lb = (batch[sel[pidx]] - g_base[c]).astype(np.int64)
        pone[c][psl, pb * P + lb] = 1.0

    # ---- edges -> cells (dst block x src residue class), padded to K*128
    pe_src = pos[s64]
    pe_dst = pos[d64]
    blk = pe_dst >> 7                                 # global dst block
    res = (pe_src & 127) >> 5
    idx16 = _bank_row(pe_src // NODES_PER_CORE,
                      (pe_src % NODES_PER_CORE) // P,
                      pe_src % P).astype(np.int16)
    ld = (pe_dst & 127).astype(np.int64)
    cell = blk * RES + res
    counts = np.bincount(cell, minlength=NBLK * RES)
    K = int(np.ceil(counts.max() / P))
    CELL = K * P
    NCHUNK = RES * K

    order = np.argsort(cell, kind="stable")
    starts = np.cumsum(counts) - counts
    within = np.arange(E) - np.repeat(starts, counts)
    flat = cell[order] * CELL + within
    idxA = np.zeros(NBLK * RES * CELL, np.int16)      # pad -> row 0 (S col 0)
    idxA[flat] = idx16[order]
    idxA = idxA.reshape(NBLK, RES, CELL)

    # host-built one-hot S: [core][p=within-chunk, (b_local*NCHUNK+rr*K+k)*P+m]
    # (pad positions simply stay zero)
    scell = np.zeros((NCORES, P, BLOCKS_PER_CORE * NCHUNK * P), F8NP)
    e_blk = blk[order]
    e_core = e_blk // BLOCKS_PER_CORE
    e_bl = e_blk % BLOCKS_PER_CORE
    e_rr = res[order]
    e_k = within // P
    e_p = within % P
    e_m = ld[order]
    col = ((e_bl * RES + e_rr) * K + e_k) * P + e_m
    scell[e_core, e_p, col] = 1.0

    # compact labels for the on-chip S build: ldc[core][p, b*NCHUNK+rr*K+k]
    ldA = np.full(NBLK * RES * CELL, -1.0, np.float16)
    ldA[flat] = ld[order].astype(np.float16)
    ldA = ldA.reshape(NBLK, RES, K, P)
    ldc = np.zeros((NCORES, P, BLOCKS_PER_CORE * NCHUNK), np.float16)
    for c in range(NCORES):
        sl = ldA[c * BLOCKS_PER_CORE:(c + 1) * BLOCKS_PER_CORE]
        ldc[c] = sl.transpose(3, 0, 1, 2).reshape(P, -1)

    # gather indices, 16-way wrapped, flat (pair-major, residue-minor)
    pairs = _pairs()
    total_cols = sum(len(pr) * B_GRP * CELL // 16 for pr in pairs) * RES
    idxw = np.zeros((NCORES, P, total_cols), np.int16)
    for c in range(NCORES):
        off = 0
        for pr in pairs:
            b0 = c * BLOCKS_PER_CORE + pr[0] * B_GRP
            nb = len(pr) * B_GRP
            for rr in range(RES):
                lst = idxA[b0:b0 + nb, rr, :].reshape(-1)
                cols = lst.shape[0] // 16
                w = lst.reshape(-1, 16).T        # slot i -> [i%16, i//16]
                idxw[c, :, off:off + cols] = np.tile(w, (8, 1))
                off += cols
        assert off == total_cols

    # ---- classifier / epilogue host data
    WnT = (cls_g[:, None] * cls_v
           / np.linalg.norm(cls_v, axis=1, keepdims=True)).T.astype(np.float16)
    cnt = np.maximum(np.bincount(batch, minlength=G).astype(np.float32), 1.0)

    trivial = dict(
        b1=not np.any(b1), b2=not np.any(b2),
        ln1=bool(np.all(ln1_w == 1.0) and not np.any(ln1_b)),
        ln2=bool(np.all(ln2_w == 1.0) and not np.any(ln2_b)),
    )
    return dict(
        K=K, xtab=xtab, xs2=xs2, d1t=d1t, pone=pone, scell=scell, ldc=ldc,
        idxw=idxw,
        WnT=WnT, cnt=cnt, g_base=g_base, trivial=trivial,
        W1h=W1.astype(np.float16), W2h=W2.astype(np.float16),
        b1=b1.astype(np.float32), b2=b2.astype(np.float32),
        ln1_w=ln1_w.astype(np.float32), ln1_b=ln1_b.astype(np.float32),
        ln2_w=ln2_w.astype(np.float32), ln2_b=ln2_b.astype(np.float32),
        cls_b=cls_b.astype(np.float32),
    )


# ---------------------------------------------------------------- program
def _build(K: int, trivial: dict, max_phase: int = 99):
    CELL = K * P
    NCHUNK = RES * K
    GH = B_GRP * H
    SGRP = B_GRP * NCHUNK * P            # S columns per group
    pairs = _pairs()
    npairs = len(pairs)
    # Shared DRAM allows only a single writer inst -> only with 1 AG chunk
    shared_tables = (not os.environ.get("BASS_LOCAL_TABLES")
                     and len(QBLK) == 1)
    # flat idx column offsets per (pair, rr)
    idx_off = {}
    off = 0
    for pi, pr in enumerate(pairs):
        cols = len(pr) * B_GRP * CELL // 16
        for rr in range(RES):
            idx_off[(pi, rr)] = (off, cols)
            off += cols
    TOTAL_IDX_COLS = off

    nc = bacc.Bacc(None, target_bir_lowering=False, debug=False,
                   num_devices=NCORES, num_swdge_queues=4)

    xtab_p = [nc.declare_dram_parameter(f"xtab{r}", [ROWS_PER_BANK, F], F16,
                                        isOutput=False) for r in range(RES)]
    xs2_p = nc.declare_dram_parameter("xs2", [P, BLOCKS_PER_CORE, F], F16,
                                      isOutput=False)
    W1_p = nc.declare_dram_parameter("W1h", [F, H], F16, isOutput=False)
    W2_p = nc.declare_dram_parameter("W2h", [H, H], F16, isOutput=False)
    idxw_p = nc.declare_dram_parameter("idxw", [P, TOTAL_IDX_COLS], I16,
                                       isOutput=False)
    s_onchip = bool(os.environ.get("BASS_S_ONCHIP"))
    single_packet = bool(os.environ.get("BASS_SP"))
    if s_onchip:
        ldc_p = nc.declare_dram_parameter(
            "ldc", [P, BLOCKS_PER_CORE * NCHUNK], F16, isOutput=False)
    else:
        scell_p = nc.declare_dram_parameter(
            "scell", [P, BLOCKS_PER_CORE * NCHUNK * P], F8, isOutput=False)
    pone_p = nc.declare_dram_parameter("pone", [P, BLOCKS_PER_CORE * P], F8,
                                       isOutput=False)
    d1t_p = nc.declare_dram_parameter("d1t", [P, BLOCKS_PER_CORE], F32,
                                      isOutput=False)
    WnT_p = nc.declare_dram_parameter("WnT", [H, C], F16, isOutput=False)
    b1_p = nc.declare_dram_parameter("b1r", [1, H], F32, isOutput=False)
    b2_p = nc.declare_dram_parameter("b2r", [1, H], F32, isOutput=False)
    ln1w_p = nc.declare_dram_parameter("ln1wr", [1, H], F32, isOutput=False)
    ln1b_p = nc.declare_dram_parameter("ln1br", [1, H], F32, isOutput=False)
    ln2w_p = nc.declare_dram_parameter("ln2wr", [1, H], F32, isOutput=False)
    ln2b_p = nc.declare_dram_parameter("ln2br", [1, H], F32, isOutput=False)
    out_p = nc.declare_dram_parameter("out_part", [P, C], F32, isOutput=True)

    with tile.TileContext(nc, num_cores=NCORES) as tc:
        with (
            tc.tile_pool(name="consts", bufs=1) as consts,
            tc.tile_pool(name="resident", bufs=1) as resident,
            tc.tile_pool(name="work", bufs=2) as work,
            tc.tile_pool(name="gat", bufs=2) as gatp,
            tc.tile_pool(name="sbp", bufs=2) as sbp,
            tc.tile_pool(name="psum_u", bufs=2, space="PSUM") as psum_u,
            tc.tile_pool(name="psum_y", bufs=2, space="PSUM") as psum_y,
            tc.tile_pool(name="psum_tr", bufs=1, space="PSUM") as psum_tr,
            tc.tile_pool(name="psum_poolg", bufs=1, space="PSUM") as psum_poolg,
            tc.tile_pool(name="dram", bufs=1, space="DRAM") as dram,
        ):
            # DRAM: layer-2 AllGather inputs (4 chunks x 4 residues) + tables
            nq = len(QBLK)
            agin = [[dram.tile([QROWS[q], H], F16, tag=f"agin{r}_{q}",
                               name=f"agin{r}_{q}") for q in range(nq)]
                    for r in range(RES)]
            tables = [dram.tile([ROWS_PER_BANK, H], F16, tag=f"tab{r}",
                                name=f"tab{r}",
                                addr_space="Shared" if shared_tables
                                else "Local")
                      for r in range(RES)]
            agin_v = [[agin[r][q][:].rearrange("(b q) d -> q b d", q=32)
                       for q in range(nq)] for r in range(RES)]

            # ---------------- constants
            W1_t = consts.tile([F, H], F16)
            nc.sync.dma_start(out=W1_t[:], in_=W1_p[:])
            W2_t = consts.tile([H, H], F16)
            nc.sync.dma_start(out=W2_t[:], in_=W2_p[:])
            d1t_t = consts.tile([P, BLOCKS_PER_CORE], F32)
            nc.sync.dma_start(out=d1t_t[:], in_=d1t_p[:])
            WnT_t = consts.tile([H, C], F16)
            nc.sync.dma_start(out=WnT_t[:], in_=WnT_p[:])
            # idx table gates the first gathers: load on the scalar queue
            idx_all = consts.tile([P, TOTAL_IDX_COLS], I16)
            nc.scalar.dma_start(out=idx_all[:], in_=idxw_p[:])
            pone_t = consts.tile([P, BLOCKS_PER_CORE * P], F8)
            nc.scalar.dma_start(out=pone_t[:], in_=pone_p[:])


            rows = {}
            for nm, pp in [("b1", b1_p), ("b2", b2_p), ("ln1w", ln1w_p),
                           ("ln1b", ln1b_p), ("ln2w", ln2w_p), ("ln2b", ln2b_p)]:
                t = consts.tile([1, H], F32, tag=f"row_{nm}")
                nc.sync.dma_start(out=t[:], in_=pp[:])
                rows[nm] = t

            ident_h = consts.tile([P, P], F16)
            make_identity(nc, ident_h[:])
            bcos_eps_t = consts.tile([P, 1], F32)
            nc.vector.memset(bcos_eps_t[:], BCOS_EPS)
            ln_eps_t = consts.tile([P, 1], F32)
            nc.vector.memset(ln_eps_t[:], LN_EPS)
            neg_one_t = consts.tile([P, 1], F32)
            nc.vector.memset(neg_one_t[:], -1.0)


            # layer-1 output table rows (dinv.*h), resident for layer 2
            hs_groups = [resident.tile([P, GH], F16, tag=f"hsg{g}",
                                       name=f"hsg{g}")
                         for g in range(N_GRP)]

            # tiny warmup AllGather: absorbs the collective cold-start
            if max_phase >= 2:
                wu_in = dram.tile([1, 8], F16, tag="wu_in", name="wu_in")
                wu_out = dram.tile([NCORES, 8], F16, tag="wu_out",
                                   name="wu_out")
                wu_s = consts.tile([1, 8], F16, tag="wu_s")
                nc.vector.memset(wu_s[:], 0.0)
                nc.sync.dma_start(out=wu_in[:], in_=wu_s[:])
                nc.gpsimd.collective_compute(
                    "AllGather", AOp.bypass,
                    replica_groups=[list(range(NCORES))],
                    ins=[wu_in[:].opt()], outs=[wu_out[:].opt()])

            gt_tiles = {}

            def issue_gathers(lyr, pi):
                pr = pairs[pi]
                nrows = len(pr) * B_GRP * CELL
                nch = nrows // P
                for rr in range(RES):
                    o, cols = idx_off[(pi, rr)]
                    gt = gatp.tile([P, 2 * B_GRP * K, H], F16,
                                   tag=f"gat{rr}", name=f"gat{rr}", bufs=3)
                    src = xtab_p[rr] if lyr == 1 else tables[rr]
                    nc.gpsimd.dma_gather(
                        out_ap=gt[:, :nch, :], in_ap=src[:],
                        idxs_ap=idx_all[:, o:o + cols],
                        num_idxs=nrows, num_idxs_reg=nrows,
                        elem_size=H, elem_step=H,
                        single_packet=single_packet,
                        queue_num=rr,
                    )
                    gt_tiles[(lyr, pi, rr)] = gt

            sg_tiles = {}

            if s_onchip:
                ldc_t = consts.tile([P, BLOCKS_PER_CORE * NCHUNK], F16)
                nc.scalar.dma_start(out=ldc_t[:], in_=ldc_p[:])
                iota_c = consts.tile([P, NCHUNK * P], F16)
                nc.gpsimd.iota(iota_c[:], pattern=[[0, NCHUNK], [1, P]],
                               base=0, channel_multiplier=0,
                               allow_small_or_imprecise_dtypes=True)

            def load_S(g, lyr=0):
                if lyr == 1:
                    own = work.tile([P, B_GRP, F], F16, tag="own", bufs=3)
                    nc.sync.dma_start(
                        out=own[:],
                        in_=xs2_p[:, g * B_GRP:(g + 1) * B_GRP, :])
                    own_map[g] = own
                if s_onchip:
                    sg = sbp.tile([P, SGRP], F8, tag="sg", name="sg", bufs=2)
                    for bl in range(B_GRP):
                        b = g * B_GRP + bl
                        nc.vector.tensor_tensor(
                            out=sg[:, bl * NCHUNK * P:(bl + 1) * NCHUNK * P]
                            .rearrange("p (c m) -> p c m", m=P),
                            in0=iota_c[:].rearrange("p (c m) -> p c m", m=P),
                            in1=ldc_t[:, b * NCHUNK:(b + 1) * NCHUNK]
                            .to_broadcast([P, NCHUNK, P]),
                            op=AOp.is_equal)
                else:
                    sg = sbp.tile([P, SGRP], F8, tag="sg", name="sg", bufs=3)
                    nc.sync.dma_start(
                        out=sg[:], in_=scell_p[:, g * SGRP:(g + 1) * SGRP])
                sg_tiles[g] = sg

            u_ps = {}
            own_map = {}

            def agg_mm(lyr, g):
                """PSUM-accumulate U^T = (S @ T)^T + T_own^T for group g."""
                pi = g // 2
                sg = sg_tiles[g]
                ups = psum_u.tile([P, GH], F32, space="PSUM", tag="u")
                u_ps[g] = ups
                bl2_0 = (g - pairs[pi][0]) * B_GRP
                for bl in range(B_GRP):
                    for rr in range(RES):
                        gt = gt_tiles[(lyr, pi, rr)]
                        for k in range(K):
                            j2 = rr * K + k
                            nc.tensor.matmul(
                                out=ups[:, bl * P:(bl + 1) * P],
                                lhsT=gt[:, (bl2_0 + bl) * K + k, :],
                                rhs=sg[:, (bl * NCHUNK + j2) * P:
                                       (bl * NCHUNK + j2 + 1) * P],
                                start=(rr == 0 and k == 0),
                                stop=False,
                            )
                    own_src = (own_map[g][:, bl, :] if lyr == 1
                               else hs_groups[g][:, bl * H:(bl + 1) * H])
                    nc.tensor.matmul(
                        out=ups[:, bl * P:(bl + 1) * P],
                        lhsT=own_src, rhs=ident_h[:],
                        start=False, stop=True,
                    )
                sg_tiles.pop(g, None)

            def finish(lyr, g, W_t, b_row, lnw_row, lnb_row, triv_b, triv_ln,
                       pool_ps):
                ups = u_ps.pop(g)
                if lyr == 1:
                    own_map.pop(g, None)
                # U^T: PSUM -> SBUF fp16 (frees the ups bank)
                u4 = work.tile([P, GH], F16, tag="u4")
                nc.scalar.activation(out=u4[:], in_=ups[:], func=Act.Copy)
                yps = psum_y.tile([P, GH], F32, space="PSUM", tag="y")
                for bl in range(B_GRP):
                    nc.tensor.matmul(out=yps[:, bl * H:(bl + 1) * H],
                                     lhsT=u4[:, bl * P:(bl + 1) * P],
                                     rhs=W_t[:], start=True, stop=True)

                gsl = slice(g * B_GRP, (g + 1) * B_GRP)
                # ---- v = (dinv_dst*) y (+ b); with b==0 the dinv_dst scale
                # is skipped: LayerNorm is invariant to per-row scaling.
                if triv_b:
                    vsrc = yps[:]
                else:
                    v4 = work.tile([P, GH], F16, tag="v4")
                    for bl in range(B_GRP):
                        nc.scalar.activation(
                            out=v4[:, bl * H:(bl + 1) * H],
                            in_=yps[:, bl * H:(bl + 1) * H], func=Act.Copy,
                            scale=d1t_t[:, g * B_GRP + bl:g * B_GRP + bl + 1])
                    nc.vector.tensor_tensor(
                        out=v4[:], in0=v4[:],
                        in1=b_row[:].to_broadcast([P, GH]), op=AOp.add)
                    vsrc = v4[:]
                # LN stats in one DVE pass; r = exp(-0.5*ln(var + eps))
                st4 = work.tile([P, B_GRP, 6], F32, tag="st4")
                mv4 = work.tile([P, B_GRP, 2], F32, tag="mv4")
                vsrc3 = vsrc.rearrange("p (b d) -> p b d", d=H)
                for bl in range(B_GRP):
                    nc.vector.bn_stats(out=st4[:, bl, :], in_=vsrc3[:, bl, :])
                    nc.vector.bn_aggr(out=mv4[:, bl, :], in_=st4[:, bl, :])
                sd4 = work.tile([P, B_GRP], F32, tag="sd4")
                nc.scalar.activation(out=sd4[:], in_=mv4[:, :, 1],
                                     func=Act.Sqrt, bias=ln_eps_t[:])
                r4 = work.tile([P, B_GRP], F32, tag="r4")
                nc.vector.reciprocal(out=r4[:], in_=sd4[:])
                mur4 = work.tile([P, B_GRP], F32, tag="mur4")
                nc.vector.tensor_tensor(out=mur4[:], in0=mv4[:, :, 0],
                                        in1=r4[:], op=AOp.mult)
                # t = v*r - mu*r  (normalized), then ELU = min(exp(t)-1,
                # relu(t))
                t4 = work.tile([P, GH], F16, tag="t4")
                nc.vector.tensor_tensor(
                    out=t4[:].rearrange("p (b d) -> p b d", d=H),
                    in0=vsrc.rearrange("p (b d) -> p b d", d=H),
                    in1=r4[:].to_broadcast([P, B_GRP, H]), op=AOp.mult)
                nc.vector.tensor_tensor(
                    out=t4[:].rearrange("p (b d) -> p b d", d=H),
                    in0=t4[:].rearrange("p (b d) -> p b d", d=H),
                    in1=mur4[:].to_broadcast([P, B_GRP, H]),
                    op=AOp.subtract)
                if not triv_ln:
                    nc.vector.tensor_tensor(
                        out=t4[:], in0=t4[:],
                        in1=lnw_row[:].to_broadcast([P, GH]), op=AOp.mult)
                    nc.vector.tensor_tensor(
                        out=t4[:], in0=t4[:],
                        in1=lnb_row[:].to_broadcast([P, GH]), op=AOp.add)
                ex4 = work.tile([P, GH], F16, tag="ex4")
                nc.scalar.activation(out=ex4[:], in_=t4[:], func=Act.Exp)
                em4 = work.tile([P, GH], F16, tag="em4")
                nc.scalar.activation(out=em4[:], in_=ex4[:],
                                     func=Act.Identity, bias=neg_one_t[:])
                rl4 = work.tile([P, GH], F16, tag="rl4")
                nc.scalar.activation(out=rl4[:], in_=t4[:], func=Act.Relu)
                h4 = work.tile([P, GH], F16, tag="h4")
                nc.vector.tensor_tensor(out=h4[:], in0=em4[:], in1=rl4[:],
                                        op=AOp.min)
                if lyr == 1:
                    # T2 rows = dinv .* h -> resident + AllGather input
                    hs4 = hs_groups[g]
                    nc.vector.tensor_tensor(
                        out=hs4[:].rearrange("p (b d) -> p b d", d=H),
                        in0=h4[:].rearrange("p (b d) -> p b d", d=H),
                        in1=d1t_t[:, gsl].to_broadcast([P, B_GRP, H]),
                        op=AOp.mult)
                    qi = QUARTER_OF_GROUP[g]
                    gb = g * B_GRP - QBLK[qi][0]
                    for r in range(RES):
                        nc.sync.dma_start(
                            out=agin_v[r][qi][:, gb:gb + B_GRP, :],
                            in_=hs4[:].rearrange("p (b d) -> p b d", d=H)
                            [32 * r:32 * (r + 1)])
                else:
                    # h_b = h * (RR + (1-RR)*TEMP / (||h|| + eps))
                    scr = work.tile([P, GH], F16, tag="scr")
                    nc.scalar.activation(out=scr[:], in_=h4[:],
                                         func=Act.Square)
                    qs4 = work.tile([P, B_GRP], F32, tag="qs4")
                    nc.vector.tensor_reduce(
                        out=qs4[:],
                        in_=scr[:].rearrange("p (b d) -> p b d", d=H),
                        axis=AxX, op=AOp.add)
                    sq4 = work.tile([P, B_GRP], F32, tag="sq4")
                    nc.scalar.activation(out=sq4[:], in_=qs4[:],
                                         func=Act.Sqrt, bias=bcos_eps_t[:])
                    sp4 = work.tile([P, B_GRP], F32, tag="sp4")
                    nc.vector.tensor_tensor(out=sp4[:], in0=sq4[:],
                                            in1=bcos_eps_t[:].to_broadcast(
                                                [P, B_GRP]), op=AOp.add)
                    rcp4 = work.tile([P, B_GRP], F32, tag="rcp4")
                    nc.vector.reciprocal(out=rcp4[:], in_=sp4[:])
                    fac4 = work.tile([P, B_GRP], F32, tag="fac4")
                    nc.scalar.activation(out=fac4[:], in_=rcp4[:],
                                         func=Act.Copy,
                                         scale=(1.0 - RR) * TEMP, bias=RR)
                    hb4 = work.tile([P, GH], F16, tag="hb4")
                    nc.vector.tensor_tensor(
                        out=hb4[:].rearrange("p (b d) -> p b d", d=H),
                        in0=h4[:].rearrange("p (b d) -> p b d", d=H),
                        in1=fac4[:].to_broadcast([P, B_GRP, H]),
                        op=AOp.mult)
                    for bl in range(B_GRP):
                        b = g * B_GRP + bl
                        nc.tensor.matmul(
                            out=pool_ps[:],
                            lhsT=pone_t[:, b * P:(b + 1) * P],
                            rhs=hb4[:, bl * H:(bl + 1) * H],
                            start=(b == 0),
                            stop=(b == BLOCKS_PER_CORE - 1))

            def fire_ag(qi):
                for r in range(RES):
                    nc.gpsimd.collective_compute(
                        "AllGather", AOp.bypass,
                        replica_groups=[list(range(NCORES))],
                        ins=[agin[r][qi][:].opt()],
                        outs=[tables[r][QBASE[qi]:
                                        QBASE[qi]
                                        + NCORES * QROWS[qi]].opt()],
                    )

            def run_layer(lyr, W_t, b_row, lnw_row, lnb_row, triv_b, triv_ln,
                          pool_ps):
                issue_gathers(lyr, 0)
                issue_gathers(lyr, 1)
                issue_gathers(lyr, 2)
                load_S(0, lyr)
                load_S(1, lyr)
                fire_points = {FIRE_AT_G[q]: q for q in range(len(QBLK))
                               if FIRE_AT_G[q] is not None}
                for g in range(N_GRP):
                    if g % 2 == 0 and g // 2 + 3 < npairs:
                        issue_gathers(lyr, g // 2 + 3)
                    if g + 2 < N_GRP:
                        load_S(g + 2, lyr)
                    agg_mm(lyr, g)
                    if lyr == 1 and g in fire_points and max_phase >= 2:
                        fire_ag(fire_points[g])
                    if g > 0:
                        finish(lyr, g - 1, W_t, b_row, lnw_row, lnb_row,
                               triv_b, triv_ln, pool_ps)
                finish(lyr, N_GRP - 1, W_t, b_row, lnw_row, lnb_row,
                       triv_b, triv_ln, pool_ps)
                if lyr == 1 and max_phase >= 2:
                    for q in range(len(QBLK)):
                        if FIRE_AT_G[q] is None:
                            fire_ag(q)

            with nc.named_scope("layer1"):
                run_layer(1, W1_t, rows["b1"], rows["ln1w"], rows["ln1b"],
                          trivial["b1"], trivial["ln1"], None)

            if max_phase < 3:
                outt0 = work.tile([P, C], F32, tag="outt")
                nc.vector.memset(outt0[:], 0.0)
                nc.sync.dma_start(out=out_p[:], in_=outt0[:])
            else:
                pool_ps = psum_poolg.tile([P, H], F32, space="PSUM")
                with nc.named_scope("layer2"):
                    run_layer(2, W2_t, rows["b2"], rows["ln2w"],
                              rows["ln2b"], trivial["b2"], trivial["ln2"],
                              pool_ps)

                # -------- pooled partial -> transpose -> classifier
                with nc.named_scope("fin"):
                    pooled = work.tile([P, H], F16, tag="pooled")
                    nc.vector.tensor_copy(out=pooled[:], in_=pool_ps[:])
                    psT = psum_tr.tile([P, P], F16, space="PSUM", tag="tr")
                    nc.tensor.transpose(out=psT[:], in_=pooled[:],
                                        identity=ident_h[:])
                    pooledT = work.tile([P, P], F16, tag="pooledT")
                    nc.vector.tensor_copy(out=pooledT[:], in_=psT[:])
                    cls_ps = psum_y.tile([P, GH], F32, space="PSUM",
                                         tag="y")
                    nc.tensor.matmul(out=cls_ps[:, :C], lhsT=pooledT[:],
                                     rhs=WnT_t[:], start=True, stop=True)
                    outt = work.tile([P, C], F32, tag="outt")
                    nc.vector.tensor_copy(out=outt[:], in_=cls_ps[:, :C])
                    nc.sync.dma_start(out=out_p[:], in_=outt[:])

    nc.finalize()
    return nc


_CACHE: dict = {}
LAST_RESULTS = None


def _ensure_ntff_hook():
    """Install the antenv.axon_hooks shim so trace=True captures NTFF
    profiles through the axon PJRT .so (the trimmed container lacks the
    module trn_boot expects)."""
    import sys as _sys
    import types

    if "antenv.axon_hooks" not in _sys.modules:
        mod = types.ModuleType("antenv.axon_hooks")
        holder = [None]
        mod.set_axon_ntff_profile_hook = lambda h: holder.__setitem__(0, h)
        mod.get_axon_ntff_profile_hook = lambda: holder[0]
        _sys.modules["antenv.axon_hooks"] = mod
        import antenv

        antenv.axon_hooks = mod
    from antenv.axon_hooks import (get_axon_ntff_profile_hook,
                                   set_axon_ntff_profile_hook)

    if get_axon_ntff_profile_hook() is None:
        from trn_agent_boot.trn_boot import _ntff_profile_via_ctypes

        h = _ntff_profile_via_ctypes("/opt/axon/libaxon_pjrt.so")
        if h is not None:
            set_axon_ntff_profile_hook(h)


def kernel(**inputs) -> np.ndarray:
    np_inputs = {k: np.asarray(v) for k, v in inputs.items()}
    prep = _prep(**np_inputs)
    K = prep["K"]
    max_phase = int(os.environ.get("BASS_MAX_PHASE", "99"))
    tkey = (K, max_phase, os.environ.get("BASS_S_ONCHIP"),
            os.environ.get("BASS_SP"), _AG_CHUNKS,
            tuple(sorted(prep["trivial"].items())))
    if tkey not in _CACHE:
        _CACHE[tkey] = _build(K, prep["trivial"], max_phase)
    nc = _CACHE[tkey]

    in_maps = []
    for c in range(NCORES):
        m = dict(
            xs2=prep["xs2"][c], W1h=prep["W1h"], W2h=prep["W2h"],
            idxw=prep["idxw"][c], d1t=prep["d1t"][c],
            pone=prep["pone"][c], WnT=prep["WnT"],
            b1r=prep["b1"][None, :], b2r=prep["b2"][None, :],
            ln1wr=prep["ln1_w"][None, :], ln1br=prep["ln1_b"][None, :],
            ln2wr=prep["ln2_w"][None, :], ln2br=prep["ln2_b"][None, :],
        )
        if os.environ.get("BASS_S_ONCHIP"):
            m["ldc"] = prep["ldc"][c]
        else:
            m["scell"] = prep["scell"][c]
        for r in range(RES):
            m[f"xtab{r}"] = prep["xtab"][r]
        in_maps.append(m)
    trace = bool(os.environ.get("BASS_KERNEL_TRACE"))
    if trace:
        _ensure_ntff_hook()
    res = run_bass_kernel_spmd(nc, in_maps, core_ids=list(range(NCORES)),
                               trace=trace)
    global LAST_RESULTS
    LAST_RESULTS = res
    if trace and res.exec_time_ns is not None:
        print(f"HW exec time: {res.exec_time_ns} ns", flush=True)

    # host unshard: scatter-add partial logits by per-core graph base,
    # divide by graph node counts, add classifier bias
    out = np.zeros((G, C), np.float64)
    for c in range(NCORES):
        part = res.results[c]["out_part"].astype(np.float64)
        gb = int(prep["g_base"][c])
        hi = min(G, gb + P)
        out[gb:hi] += part[: hi - gb]
    out = out / prep["cnt"][:, None] + prep["cls_b"][None, :]
    return out.astype(np.float32)
